# revision 5
# baseline (speedup 1.0000x reference)
"""Multi-head self-attention (L=2048, N=4, E=1024, h=16) on 8 NeuronCores.

Sharding: core c handles batch n = c//2 and heads [8*(c%2), 8*(c%2)+8).
Each core computes q/k/v projections for its (n, head-block), attention,
and a partial out-projection (columns of out_proj for its heads).
Host sums the two bf16 partials per batch n and adds out_bias.

v2 schedule: one uniform pipeline over 128 global key-steps (8 chunks x
16 lk).  Per step: QK pair + exp per lane; pv/den matmuls lag 3 steps
and carry across chunk boundaries (no boundary stall); each chunk's
normalization tail (strided 4-row reciprocal -> stage -> gpsimd
partition_broadcast -> mults) runs decoupled on DVE/GPSIMD.  The qt
DMA streams in [128,512] pieces (ch0 first) so the first exp fires
~15us in; projection/out-proj work is spread as deadline-scheduled
fillers (v-proj split by round, qT23 + out-proj inside round 1).

Known HW pitfalls encoded here: reciprocal_approx_fast (custom-DVE)
computes garbage via this compile path; partition_broadcast requires
partition-0 sources; GPSIMD cannot access PSUM; interleaved PSUM
accumulation groups need skip_group_check (per-element has_written).
"""

from collections import defaultdict
from contextlib import ExitStack

import ml_dtypes
import numpy as np

import concourse.bacc as bacc
import concourse.mybir as mybir
import concourse.tile as tile
from concourse.bass_utils import run_bass_kernel_spmd

L, N, E, H, D = 2048, 4, 1024, 16, 64
SCALE = D**-0.5
IL = 512  # inner dims per core (8 heads * 64)
P = 128
F32 = mybir.dt.float32
BF16 = mybir.dt.bfloat16
EXP = mybir.ActivationFunctionType.Exp

_built = None


def build(dbg=False):
    nc = bacc.Bacc("TRN2", target_bir_lowering=False, debug=False, num_devices=8)

    qt_d = nc.dram_tensor("qt", [E, L], BF16, kind="ExternalInput")
    wq_d = nc.dram_tensor("wq", [E, IL], BF16, kind="ExternalInput")
    wk_d = nc.dram_tensor("wk", [E, IL], BF16, kind="ExternalInput")
    wv_d = nc.dram_tensor("wv", [E, IL], BF16, kind="ExternalInput")
    bq_d = nc.dram_tensor("bq", [4, P], F32, kind="ExternalInput")
    bk_d = nc.dram_tensor("bk", [4, P], F32, kind="ExternalInput")
    bvb_d = nc.dram_tensor("bvb", [P, IL], F32, kind="ExternalInput")
    opt_d = nc.dram_tensor("opt", [IL, E], BF16, kind="ExternalInput")
    out_d = nc.dram_tensor("out", [L, E], BF16, kind="ExternalOutput")

    with tile.TileContext(nc) as tc:
        est = ExitStack()
        persist = est.enter_context(tc.tile_pool(name="persist", bufs=1))

        ones_col = persist.tile([P, 1], BF16, name="ones_col")
        nc.vector.memset(ones_col, 1.0)

        qT = [persist.tile([P, L], BF16, name=f"qT{m}") for m in range(4)]
        kT = [persist.tile([P, L], BF16, name=f"kT{m}") for m in range(4)]
        vv = [persist.tile([P, IL], BF16, name=f"v{t}") for t in range(16)]
        aoT = [persist.tile([P, L], BF16, name=f"aoT{m}") for m in range(4)]
        opt_sb = [persist.tile([P, E], BF16, name=f"opt{k}") for k in range(4)]
        qt_sb = [persist.tile([P, L], BF16, name=f"qtsb{t}") for t in range(8)]
        wq_sb = [persist.tile([P, IL], BF16, name=f"wq{t}") for t in range(8)]
        wk_sb = [persist.tile([P, IL], BF16, name=f"wk{t}") for t in range(8)]
        wv_sb = [persist.tile([P, IL], BF16, name=f"wv{t}") for t in range(8)]
        bq_sb = persist.tile([P, 4], F32, name="bq_sb")
        bk_sb = persist.tile([P, 4], F32, name="bk_sb")
        bvb_sb = persist.tile([P, IL], F32, name="bvb_sb")

        # ---------------- streaming inputs ----------------
        # Issue order = consumption order: wk + qt-ch0 feed the kT01ch0
        # prologue, wq the qT01ch0 prologue; later qt chunks arrive while
        # the pipeline runs; wv/bvb before the chunk-0 v fillers.
        def dma_qt_ch(ch):
            for t in range(8):
                nc.sync.dma_start(
                    out=qt_sb[t][:, ch * 512 : (ch + 1) * 512],
                    in_=qt_d[t * P : (t + 1) * P, ch * 512 : (ch + 1) * 512],
                )

        for t in range(8):
            nc.sync.dma_start(out=wk_sb[t], in_=wk_d[t * P : (t + 1) * P, :])
        for m in range(4):
            nc.sync.dma_start(out=bk_sb[:, m : m + 1], in_=bk_d[m, :, None])
            nc.sync.dma_start(out=bq_sb[:, m : m + 1], in_=bq_d[m, :, None])
        dma_qt_ch(0)
        for t in range(8):
            nc.sync.dma_start(out=wq_sb[t], in_=wq_d[t * P : (t + 1) * P, :])
        dma_qt_ch(1)
        for t in range(8):
            nc.sync.dma_start(out=wv_sb[t], in_=wv_d[t * P : (t + 1) * P, :])
        nc.sync.dma_start(out=bvb_sb, in_=bvb_d[:, :])
        dma_qt_ch(2)
        dma_qt_ch(3)
        for k in range(4):
            nc.sync.dma_start(out=opt_sb[k], in_=opt_d[k * P : (k + 1) * P, :])

        # ---------------- prologue: kT01 ch0 + qT01 ch0 (DMA-paced) -----
        with tc.tile_pool(name="pro_ps", bufs=1, space="PSUM") as pro_ps:
            psk = {
                m: pro_ps.tile([P, 512], F32, tag=f"pk{m}", name=f"psk{m}")
                for m in (0, 1)
            }
            for t in range(8):
                for m in (0, 1):
                    nc.tensor.matmul(
                        psk[m],
                        wk_sb[t][:, m * P : (m + 1) * P],
                        qt_sb[t][:, 0:512],
                        start=(t == 0),
                        stop=(t == 7),
                    )
            for m in (0, 1):
                nc.vector.tensor_scalar_add(
                    out=kT[m][:, 0:512], in0=psk[m], scalar1=bk_sb[:, m : m + 1]
                )
            psq = {
                m: pro_ps.tile([P, 512], F32, tag=f"pq{m}", name=f"psq{m}")
                for m in (0, 1)
            }
            for t in range(8):
                for m in (0, 1):
                    nc.tensor.matmul(
                        psq[m],
                        wq_sb[t][:, m * P : (m + 1) * P],
                        qt_sb[t][:, 0:512],
                        start=(t == 0),
                        stop=(t == 7),
                    )
            for m in (0, 1):
                nc.vector.tensor_scalar_add(
                    out=qT[m][:, 0:512], in0=psq[m], scalar1=bq_sb[:, m : m + 1]
                )

        # ---------------- pipeline pools ----------------
        ph2 = est.enter_context(ExitStack())
        at_pools = [
            ph2.enter_context(tc.tile_pool(name=f"at{i}", bufs=4)) for i in (0, 1)
        ]
        small = ph2.enter_context(tc.tile_pool(name="small", bufs=4))
        osb = ph2.enter_context(tc.tile_pool(name="osb", bufs=3))
        pvc = ph2.enter_context(tc.tile_pool(name="pvc", bufs=4))
        st_ps = [
            ph2.enter_context(tc.tile_pool(name=f"st{i}", bufs=1, space="PSUM"))
            for i in (0, 1)
        ]
        pv_ps = [
            ph2.enter_context(tc.tile_pool(name=f"pv{i}", bufs=1, space="PSUM"))
            for i in (0, 1)
        ]
        den_ps = ph2.enter_context(tc.tile_pool(name="den", bufs=1, space="PSUM"))
        fill_ps = ph2.enter_context(tc.tile_pool(name="fill", bufs=1, space="PSUM"))

        # shared den tile: rows 64i+32j.  Each tail copies those 4 rows to
        # dcp (same-partition, cheap) so the next chunk's den matmuls WAR
        # only on the tiny copies, not the ~3us reciprocal; dcp's other
        # rows are memset-armed once so the [0:97] reciprocal reads
        # defined data.
        den_t = den_ps.tile([P, 512], F32, name="den_g")
        dcp = persist.tile([P, 512], F32, name="dcp")
        nc.vector.memset(dcp, 1.0)

        ats_hist = {}  # global step g -> [at tiles per lane]
        pv_cur = {}  # lane p -> psum tile of the in-flight chunk

        def lanes_of(c):
            return (2 * (c // 4), 2 * (c // 4) + 1)

        def qk_exp(c, lk, g):
            rnd, lq = divmod(c, 4)
            lqs = slice(lq * 512, (lq + 1) * 512)
            lks = slice(lk * P, (lk + 1) * P)
            ats = []
            for i, p in enumerate(lanes_of(c)):
                st = st_ps[i].tile([P, 2, 512], F32, tag="st", name=f"st_{p}_{g}")
                for j in (0, 1):
                    nc.tensor.matmul(
                        st[:, j, :],
                        kT[p][64 * j : 64 * j + 64, lks],
                        qT[p][64 * j : 64 * j + 64, lqs],
                        start=True,
                        stop=True,
                    )
                at = at_pools[i].tile(
                    [P, 2, 512], BF16, tag="at", name=f"at_{p}_{g}"
                )
                nc.scalar.activation(out=at, in_=st, func=EXP)
                ats.append(at)
            ats_hist[g] = ats

        def pv_den(g2):
            c2, lk2 = divmod(g2, 16)
            lanes = lanes_of(c2)
            ats = ats_hist.pop(g2)
            if lk2 == 0:
                for i, p in enumerate(lanes):
                    pv_cur[p] = pv_ps[i].tile(
                        [P, 512], F32, tag="pv", name=f"pv_{p}_{c2}"
                    )
            for i, p in enumerate(lanes):
                for j in (0, 1):
                    nc.tensor.matmul(
                        pv_cur[p][64 * j : 64 * j + 64, :],
                        vv[lk2][:, P * p + 64 * j : P * p + 64 * j + 64],
                        ats[i][:, j, :],
                        start=(lk2 == 0),
                        stop=(lk2 == 15),
                        skip_group_check=True,
                    )
            for i, p in enumerate(lanes):
                for j in (0, 1):
                    r0 = 64 * i + 32 * j
                    nc.tensor.matmul(
                        den_t[r0 : r0 + 1, :],
                        ones_col,
                        ats[i][:, j, :],
                        start=(lk2 == 0),
                        stop=(lk2 == 15),
                        tile_position=(0, r0),
                        skip_group_check=True,
                    )

        def tail(c2, last=False):
            rnd, lq = divmod(c2, 4)
            lanes = lanes_of(c2)
            lqs = slice(lq * 512, (lq + 1) * 512)
            for i, p in enumerate(lanes):
                for j in (0, 1):
                    r0 = 64 * i + 32 * j
                    nc.vector.tensor_copy(
                        out=dcp[r0 : r0 + 1, :], in_=den_t[r0 : r0 + 1, :]
                    )
            rcp = pvc.tile([P, 512], F32, tag="rcp", name=f"rcp_{c2}", bufs=2)
            nc.vector.reciprocal(out=rcp[0:97, :], in_=dcp[0:97, :])
            if not last:
                pvs = {}
                for i, p in enumerate(lanes):
                    pvs[p] = pvc.tile([P, 512], F32, tag="pvc", name=f"pvc_{p}_{c2}")
                    nc.vector.tensor_copy(out=pvs[p], in_=pv_cur[p])
            else:
                pvs = {p: pv_cur[p] for p in lanes}
            for i, p in enumerate(lanes):
                bcs = small.tile(
                    [P, 2, 512], F32, tag="bcs", name=f"bcs_{p}_{c2}", bufs=2
                )
                rc = small.tile(
                    [1, 2, 512], F32, tag="rc", name=f"rc_{p}_{c2}", bufs=2
                )
                # partition_broadcast's ucode reads via Q7 core 0 only,
                # so the source must sit on partition 0 — stage the two
                # reciprocal rows there first.
                for j in (0, 1):
                    r0 = 64 * i + 32 * j
                    nc.vector.tensor_copy(out=rc[:, j, :], in_=rcp[r0 : r0 + 1, :])
                nc.gpsimd.partition_broadcast(bcs, rc)
                for j in (0, 1):
                    nc.vector.tensor_mul(
                        out=aoT[p][64 * j : 64 * j + 64, lqs],
                        in0=pvs[p][64 * j : 64 * j + 64, :],
                        in1=bcs[64 * j : 64 * j + 64, j, :],
                    )

        # ---------------- deadline-scheduled fillers ----------------
        sched = defaultdict(list)

        def v_piece(lk, rnd):
            def thunk():
                ps = fill_ps.tile([P, 512], F32, tag="fill", name=f"fv{lk}{rnd}")
                cs = slice(rnd * 256, rnd * 256 + 256)
                for t in range(8):
                    nc.tensor.matmul(
                        ps[:, 0:256],
                        qt_sb[t][:, lk * P : (lk + 1) * P],
                        wv_sb[t][:, cs],
                        start=(t == 0),
                        stop=(t == 7),
                    )
                nc.vector.tensor_add(
                    out=vv[lk][:, cs], in0=ps[:, 0:256], in1=bvb_sb[:, cs]
                )
            return thunk

        def proj_piece(dest, w_sb, bias_sb, m, ch, nm):
            def thunk():
                ps = fill_ps.tile([P, 512], F32, tag="fill", name=f"f{nm}{m}{ch}")
                for t in range(8):
                    nc.tensor.matmul(
                        ps,
                        w_sb[t][:, m * P : (m + 1) * P],
                        qt_sb[t][:, ch * 512 : (ch + 1) * 512],
                        start=(t == 0),
                        stop=(t == 7),
                    )
                nc.vector.tensor_scalar_add(
                    out=dest[m][:, ch * 512 : (ch + 1) * 512],
                    in0=ps,
                    scalar1=bias_sb[:, m : m + 1],
                )
            return thunk

        def outproj_piece(lt, cc, act_evac=False):
            def thunk():
                ps = fill_ps.tile([P, 512], F32, tag="fill", name=f"fo{lt}{cc}")
                for k in range(4):
                    nc.tensor.matmul(
                        ps,
                        aoT[k][:, lt * P : (lt + 1) * P],
                        opt_sb[k][:, cc * 512 : (cc + 1) * 512],
                        start=(k == 0),
                        stop=(k == 3),
                    )
                ob = osb.tile([P, 512], BF16, tag="ob", name=f"ob{lt}{cc}")
                if act_evac:
                    nc.scalar.copy(out=ob, in_=ps)
                else:
                    nc.vector.tensor_copy(out=ob, in_=ps)
                nc.sync.dma_start(
                    out=out_d[lt * P : (lt + 1) * P, cc * 512 : (cc + 1) * 512],
                    in_=ob,
                )
            return thunk

        # v projection: round-half r during the first chunk of round r.
        for rnd in (0, 1):
            for lk in range(16):
                sched[64 * rnd + lk].append(v_piece(lk, rnd))
        # kT01 remaining chunks: ch needed by chunk-0 QK at lk = 4*ch.
        for ch, gs in ((1, (0, 1)), (2, (4, 5)), (3, (8, 9))):
            for m, g in zip((0, 1), gs):
                sched[g].append(proj_piece(kT, wk_sb, bk_sb, m, ch, "k"))
        # qT01 ch1..3: needed at the start of round-0 chunk ch.
        for ch, gs in ((1, (12, 13)), (2, (20, 21)), (3, (36, 37))):
            for m, g in zip((0, 1), gs):
                sched[g].append(proj_piece(qT, wq_sb, bq_sb, m, ch, "q"))
        # kT23 (full): needed by round-1 chunk 0 (g=64).
        for (m, ch), g in zip(
            [(m, ch) for ch in range(4) for m in (2, 3)],
            (24, 25, 28, 29, 40, 41, 44, 45),
        ):
            sched[g].append(proj_piece(kT, wk_sb, bk_sb, m, ch, "k"))
        # qT23: ch c needed at the start of round-1 chunk c.
        for ch, gs in ((0, (52, 53)), (1, (60, 61)), (2, (82, 83)), (3, (98, 99))):
            for m, g in zip((2, 3), gs):
                sched[g].append(proj_piece(qT, wq_sb, bq_sb, m, ch, "q"))
        # out-projection: rows lt ready after round-1 chunk lt//4's tail.
        for lt0, g0 in ((0, 84), (4, 100), (8, 116)):
            gg = g0
            for lt in range(lt0, lt0 + 4):
                for cc in (0, 1):
                    sched[gg].append(outproj_piece(lt, cc))
                    gg += 1
        drain = [outproj_piece(lt, cc, act_evac=True) for lt in range(12, 16) for cc in (0, 1)]

        # ---------------- main pipeline ----------------
        for g in range(128):
            c, lk = divmod(g, 16)
            qk_exp(c, lk, g)
            if g >= 3:
                pv_den(g - 3)
                if (g - 3) % 16 == 15:
                    tail((g - 3) // 16)
            for thunk in sched.pop(g, ()):
                thunk()
        for g2 in (125, 126, 127):
            pv_den(g2)
        tail(7, last=True)
        for thunk in drain:
            thunk()
        assert not sched and not ats_hist

        est.close()

    nc.compile()
    return nc


def _prep_inputs(query, qkv_proj, qkv_bias, out_proj):
    """Per-core input shards (host-side)."""
    query = np.asarray(query, dtype=np.float32)
    qkv_proj = np.asarray(qkv_proj, dtype=np.float32)
    qkv_bias = np.asarray(qkv_bias, dtype=np.float32)
    W3 = qkv_proj.reshape(E, 3, E)  # [i, c, e], row f = 3*i + c
    b3 = qkv_bias.reshape(E, 3)
    bf = ml_dtypes.bfloat16
    maps = []
    for c in range(8):
        n, half = c // 2, c % 2
        isl = slice(IL * half, IL * half + IL)
        maps.append(
            {
                "qt": np.ascontiguousarray(query[:, n, :].T).astype(bf),
                "wq": np.ascontiguousarray(W3[isl, 0, :].T * SCALE).astype(bf),
                "wk": np.ascontiguousarray(W3[isl, 1, :].T).astype(bf),
                "wv": np.ascontiguousarray(W3[isl, 2, :].T).astype(bf),
                "bq": np.ascontiguousarray((b3[isl, 0] * SCALE).reshape(4, P)),
                "bk": np.ascontiguousarray(b3[isl, 1].reshape(4, P)),
                "bvb": np.ascontiguousarray(np.broadcast_to(b3[isl, 2], (P, IL))),
                "opt": np.ascontiguousarray(out_proj[:, isl].T).astype(bf),
            }
        )
    return maps


def kernel(query, qkv_proj, qkv_bias, out_proj, out_bias, **run_kwargs):
    global _built
    out_proj = np.asarray(out_proj, dtype=np.float32)
    out_bias = np.asarray(out_bias, dtype=np.float32)
    if _built is None:
        _built = build()
    in_maps = _prep_inputs(query, qkv_proj, qkv_bias, out_proj)
    res = run_bass_kernel_spmd(_built, in_maps, core_ids=list(range(8)), **run_kwargs)
    parts = [r["out"].astype(np.float32) for r in res.results]
    out = np.empty((L, N, E), dtype=np.float32)
    for n in range(N):
        out[:, n, :] = parts[2 * n] + parts[2 * n + 1] + out_bias
    kernel.last_result = res
    return out


# revision 8
# speedup vs baseline: 1.1998x; 1.1998x over previous
"""Multi-head self-attention (L=2048, N=4, E=1024, h=16) on 8 NeuronCores.

Sharding: core c handles batch n = c//2 and heads [8*(c%2), 8*(c%2)+8).
Each core computes q/k/v projections for its (n, head-block), attention,
and a partial out-projection (columns of out_proj for its heads).
Host sums the two bf16 partials per batch n and adds out_bias.

v2 schedule: one uniform pipeline over 128 global key-steps (8 chunks x
16 lk).  Per step: QK pair + exp per lane; pv/den matmuls lag 3 steps
and carry across chunk boundaries (no boundary stall); each chunk's
normalization tail (strided 4-row reciprocal -> stage -> gpsimd
partition_broadcast -> mults) runs decoupled on DVE/GPSIMD.  The qt
DMA streams in [128,512] pieces (ch0 first) so the first exp fires
~15us in; projection/out-proj work is spread as deadline-scheduled
fillers (v-proj split by round, qT23 + out-proj inside round 1).

Known HW pitfalls encoded here: reciprocal_approx_fast (custom-DVE)
computes garbage via this compile path; partition_broadcast requires
partition-0 sources; GPSIMD cannot access PSUM; interleaved PSUM
accumulation groups need skip_group_check (per-element has_written).
"""

from collections import defaultdict
from contextlib import ExitStack

import ml_dtypes
import numpy as np

import concourse.bacc as bacc
import concourse.mybir as mybir
import concourse.tile as tile
from concourse.bass_utils import run_bass_kernel_spmd

L, N, E, H, D = 2048, 4, 1024, 16, 64
SCALE = D**-0.5
IL = 512  # inner dims per core (8 heads * 64)
P = 128
F32 = mybir.dt.float32
BF16 = mybir.dt.bfloat16
EXP = mybir.ActivationFunctionType.Exp

_built = None


def build(dbg=False):
    nc = bacc.Bacc("TRN2", target_bir_lowering=False, debug=False, num_devices=8)

    qt_d = nc.dram_tensor("qt", [E, L], BF16, kind="ExternalInput")
    wq_d = nc.dram_tensor("wq", [E, IL], BF16, kind="ExternalInput")
    wk_d = nc.dram_tensor("wk", [E, IL], BF16, kind="ExternalInput")
    wv_d = nc.dram_tensor("wv", [E, IL], BF16, kind="ExternalInput")
    bq_d = nc.dram_tensor("bq", [4, P], F32, kind="ExternalInput")
    bk_d = nc.dram_tensor("bk", [4, P], F32, kind="ExternalInput")
    bvb_d = nc.dram_tensor("bvb", [P, IL], F32, kind="ExternalInput")
    opt_d = nc.dram_tensor("opt", [IL, E], BF16, kind="ExternalInput")
    out_d = nc.dram_tensor("out", [L, E], BF16, kind="ExternalOutput")

    with tile.TileContext(nc) as tc:
        est = ExitStack()
        persist = est.enter_context(tc.tile_pool(name="persist", bufs=1))

        ones_col = persist.tile([P, 1], BF16, name="ones_col")
        nc.vector.memset(ones_col, 1.0)

        qT = [persist.tile([P, L], BF16, name=f"qT{m}") for m in range(4)]
        kT = [persist.tile([P, L], BF16, name=f"kT{m}") for m in range(4)]
        vv = [persist.tile([P, IL], BF16, name=f"v{t}") for t in range(16)]
        aoT = [persist.tile([P, L], BF16, name=f"aoT{m}") for m in range(4)]
        opt_sb = [persist.tile([P, E], BF16, name=f"opt{k}") for k in range(4)]
        qt_sb = [persist.tile([P, L], BF16, name=f"qtsb{t}") for t in range(8)]
        wq_sb = [persist.tile([P, IL], BF16, name=f"wq{t}") for t in range(8)]
        wk_sb = [persist.tile([P, IL], BF16, name=f"wk{t}") for t in range(8)]
        wv_sb = [persist.tile([P, IL], BF16, name=f"wv{t}") for t in range(8)]
        bq_sb = persist.tile([P, 4], F32, name="bq_sb")
        bk_sb = persist.tile([P, 4], F32, name="bk_sb")
        bvb_sb = persist.tile([P, IL], F32, name="bvb_sb")

        # ---------------- streaming inputs ----------------
        # Issue order = consumption order: wk + qt-ch0 feed the kT01ch0
        # prologue, wq the qT01ch0 prologue; later qt chunks arrive while
        # the pipeline runs; wv/bvb before the chunk-0 v fillers.
        def dma_qt_ch(ch):
            for t in range(8):
                nc.sync.dma_start(
                    out=qt_sb[t][:, ch * 512 : (ch + 1) * 512],
                    in_=qt_d[t * P : (t + 1) * P, ch * 512 : (ch + 1) * 512],
                )

        for t in range(8):
            nc.sync.dma_start(out=wk_sb[t], in_=wk_d[t * P : (t + 1) * P, :])
            nc.sync.dma_start(
                out=qt_sb[t][:, 0:512], in_=qt_d[t * P : (t + 1) * P, 0:512]
            )
        for m in range(4):
            nc.sync.dma_start(out=bk_sb[:, m : m + 1], in_=bk_d[m, :, None])
            nc.sync.dma_start(out=bq_sb[:, m : m + 1], in_=bq_d[m, :, None])
        for t in range(8):
            nc.sync.dma_start(out=wq_sb[t], in_=wq_d[t * P : (t + 1) * P, :])
        dma_qt_ch(1)
        for t in range(8):
            nc.sync.dma_start(out=wv_sb[t], in_=wv_d[t * P : (t + 1) * P, :])
        nc.sync.dma_start(out=bvb_sb, in_=bvb_d[:, :])
        dma_qt_ch(2)
        dma_qt_ch(3)
        for k in range(4):
            nc.sync.dma_start(out=opt_sb[k], in_=opt_d[k * P : (k + 1) * P, :])

        # ---------------- prologue: kT01 ch0 + qT01 ch0 (DMA-paced) -----
        with tc.tile_pool(name="pro_ps", bufs=1, space="PSUM") as pro_ps:
            psk = {
                m: pro_ps.tile([P, 512], F32, tag=f"pk{m}", name=f"psk{m}")
                for m in (0, 1)
            }
            for t in range(8):
                for m in (0, 1):
                    nc.tensor.matmul(
                        psk[m],
                        wk_sb[t][:, m * P : (m + 1) * P],
                        qt_sb[t][:, 0:512],
                        start=(t == 0),
                        stop=(t == 7),
                    )
            for m in (0, 1):
                nc.vector.tensor_scalar_add(
                    out=kT[m][:, 0:512], in0=psk[m], scalar1=bk_sb[:, m : m + 1]
                )
            psq = {
                m: pro_ps.tile([P, 512], F32, tag=f"pq{m}", name=f"psq{m}")
                for m in (0, 1)
            }
            for t in range(8):
                for m in (0, 1):
                    nc.tensor.matmul(
                        psq[m],
                        wq_sb[t][:, m * P : (m + 1) * P],
                        qt_sb[t][:, 0:512],
                        start=(t == 0),
                        stop=(t == 7),
                    )
            for m in (0, 1):
                nc.vector.tensor_scalar_add(
                    out=qT[m][:, 0:512], in0=psq[m], scalar1=bq_sb[:, m : m + 1]
                )

        # ---------------- pipeline pools ----------------
        ph2 = est.enter_context(ExitStack())
        LAG = 4  # pv/den lag behind QK/exp, in global steps
        at_pools = [
            ph2.enter_context(tc.tile_pool(name=f"at{i}", bufs=LAG + 1))
            for i in (0, 1)
        ]
        small = ph2.enter_context(tc.tile_pool(name="small", bufs=4))
        osb = ph2.enter_context(tc.tile_pool(name="osb", bufs=3))
        pvc = ph2.enter_context(tc.tile_pool(name="pvc", bufs=4))
        st_ps = [
            ph2.enter_context(tc.tile_pool(name=f"st{i}", bufs=1, space="PSUM"))
            for i in (0, 1)
        ]
        pv_ps = [
            ph2.enter_context(tc.tile_pool(name=f"pv{i}", bufs=1, space="PSUM"))
            for i in (0, 1)
        ]
        den_ps = ph2.enter_context(tc.tile_pool(name="den", bufs=1, space="PSUM"))
        fill_ps = ph2.enter_context(tc.tile_pool(name="fill", bufs=1, space="PSUM"))

        # shared den tile: rows 64i+32j.  Each tail copies those 4 rows to
        # dcp (same-partition, cheap) so the next chunk's den matmuls WAR
        # only on the tiny copies, not the ~3us reciprocal; dcp's other
        # rows are memset-armed once so the [0:97] reciprocal reads
        # defined data.
        den_t = den_ps.tile([P, 512], F32, name="den_g")
        dcp = persist.tile([P, 512], F32, name="dcp")
        nc.vector.memset(dcp, 1.0)

        ats_hist = {}  # global step g -> [at tiles per lane]
        pv_cur = {}  # lane p -> psum tile of the in-flight chunk

        def lanes_of(c):
            return (2 * (c // 4), 2 * (c // 4) + 1)

        def qk_exp(c, lk, g):
            rnd, lq = divmod(c, 4)
            lqs = slice(lq * 512, (lq + 1) * 512)
            lks = slice(lk * P, (lk + 1) * P)
            ats = []
            for i, p in enumerate(lanes_of(c)):
                st = st_ps[i].tile([P, 2, 512], F32, tag="st", name=f"st_{p}_{g}")
                for j in (0, 1):
                    nc.tensor.matmul(
                        st[:, j, :],
                        kT[p][64 * j : 64 * j + 64, lks],
                        qT[p][64 * j : 64 * j + 64, lqs],
                        start=True,
                        stop=True,
                    )
                at = at_pools[i].tile(
                    [P, 2, 512], BF16, tag="at", name=f"at_{p}_{g}"
                )
                nc.scalar.activation(out=at, in_=st, func=EXP)
                ats.append(at)
            ats_hist[g] = ats

        def pv_den(g2):
            c2, lk2 = divmod(g2, 16)
            lanes = lanes_of(c2)
            ats = ats_hist.pop(g2)
            if lk2 == 0:
                for i, p in enumerate(lanes):
                    pv_cur[p] = pv_ps[i].tile(
                        [P, 512], F32, tag="pv", name=f"pv_{p}_{c2}"
                    )
            for i, p in enumerate(lanes):
                for j in (0, 1):
                    nc.tensor.matmul(
                        pv_cur[p][64 * j : 64 * j + 64, :],
                        vv[lk2][:, P * p + 64 * j : P * p + 64 * j + 64],
                        ats[i][:, j, :],
                        start=(lk2 == 0),
                        stop=(lk2 == 15),
                        skip_group_check=True,
                    )
            for i, p in enumerate(lanes):
                for j in (0, 1):
                    r0 = 64 * i + 32 * j
                    nc.tensor.matmul(
                        den_t[r0 : r0 + 1, :],
                        ones_col,
                        ats[i][:, j, :],
                        start=(lk2 == 0),
                        stop=(lk2 == 15),
                        tile_position=(0, r0),
                        skip_group_check=True,
                    )

        def tail(c2, last=False):
            rnd, lq = divmod(c2, 4)
            lanes = lanes_of(c2)
            lqs = slice(lq * 512, (lq + 1) * 512)
            # DVE order matters: the pv copies and den-row copies free the
            # PSUM banks the next chunk's pv/den matmuls WAR on, so they
            # must precede the ~3.5us reciprocal in the DVE FIFO.
            if not last:
                pvs = {}
                for i, p in enumerate(lanes):
                    pvs[p] = pvc.tile([P, 512], F32, tag="pvc", name=f"pvc_{p}_{c2}")
                    nc.vector.tensor_copy(out=pvs[p], in_=pv_cur[p])
            else:
                pvs = {p: pv_cur[p] for p in lanes}
            for i, p in enumerate(lanes):
                for j in (0, 1):
                    r0 = 64 * i + 32 * j
                    nc.vector.tensor_copy(
                        out=dcp[r0 : r0 + 1, :], in_=den_t[r0 : r0 + 1, :]
                    )
            rcp = pvc.tile([P, 512], F32, tag="rcp", name=f"rcp_{c2}", bufs=2)
            nc.vector.reciprocal(out=rcp[0:97, :], in_=dcp[0:97, :])
            for i, p in enumerate(lanes):
                bcs = small.tile(
                    [P, 2, 512], F32, tag="bcs", name=f"bcs_{p}_{c2}", bufs=2
                )
                rc = small.tile(
                    [1, 2, 512], F32, tag="rc", name=f"rc_{p}_{c2}", bufs=2
                )
                # partition_broadcast's ucode reads via Q7 core 0 only,
                # so the source must sit on partition 0 — stage the two
                # reciprocal rows there first.
                for j in (0, 1):
                    r0 = 64 * i + 32 * j
                    nc.vector.tensor_copy(out=rc[:, j, :], in_=rcp[r0 : r0 + 1, :])
                nc.gpsimd.partition_broadcast(bcs, rc)
                for j in (0, 1):
                    nc.vector.tensor_mul(
                        out=aoT[p][64 * j : 64 * j + 64, lqs],
                        in0=pvs[p][64 * j : 64 * j + 64, :],
                        in1=bcs[64 * j : 64 * j + 64, j, :],
                    )

        # ---------------- deadline-scheduled fillers ----------------
        sched = defaultdict(list)

        def v_piece(lk, rnd):
            def thunk():
                ps = fill_ps.tile([P, 512], F32, tag="fill", name=f"fv{lk}{rnd}")
                cs = slice(rnd * 256, rnd * 256 + 256)
                for t in range(8):
                    nc.tensor.matmul(
                        ps[:, 0:256],
                        qt_sb[t][:, lk * P : (lk + 1) * P],
                        wv_sb[t][:, cs],
                        start=(t == 0),
                        stop=(t == 7),
                    )
                nc.vector.tensor_add(
                    out=vv[lk][:, cs], in0=ps[:, 0:256], in1=bvb_sb[:, cs]
                )
            return thunk

        def proj_piece(dest, w_sb, bias_sb, m, ch, nm):
            def thunk():
                ps = fill_ps.tile([P, 512], F32, tag="fill", name=f"f{nm}{m}{ch}")
                for t in range(8):
                    nc.tensor.matmul(
                        ps,
                        w_sb[t][:, m * P : (m + 1) * P],
                        qt_sb[t][:, ch * 512 : (ch + 1) * 512],
                        start=(t == 0),
                        stop=(t == 7),
                    )
                nc.vector.tensor_scalar_add(
                    out=dest[m][:, ch * 512 : (ch + 1) * 512],
                    in0=ps,
                    scalar1=bias_sb[:, m : m + 1],
                )
            return thunk

        def outproj_piece(lt, cc, act_evac=False):
            def thunk():
                ps = fill_ps.tile([P, 512], F32, tag="fill", name=f"fo{lt}{cc}")
                for k in range(4):
                    nc.tensor.matmul(
                        ps,
                        aoT[k][:, lt * P : (lt + 1) * P],
                        opt_sb[k][:, cc * 512 : (cc + 1) * 512],
                        start=(k == 0),
                        stop=(k == 3),
                    )
                ob = osb.tile([P, 512], BF16, tag="ob", name=f"ob{lt}{cc}")
                if act_evac:
                    nc.scalar.copy(out=ob, in_=ps)
                else:
                    nc.vector.tensor_copy(out=ob, in_=ps)
                nc.sync.dma_start(
                    out=out_d[lt * P : (lt + 1) * P, cc * 512 : (cc + 1) * 512],
                    in_=ob,
                )
            return thunk

        # v projection: round-half r during the first chunk of round r.
        for rnd in (0, 1):
            for lk in range(16):
                sched[64 * rnd + lk].append(v_piece(lk, rnd))
        # kT01 remaining chunks: ch needed by chunk-0 QK at lk = 4*ch.
        for ch, gs in ((1, (0, 1)), (2, (4, 5)), (3, (8, 9))):
            for m, g in zip((0, 1), gs):
                sched[g].append(proj_piece(kT, wk_sb, bk_sb, m, ch, "k"))
        # qT01 ch1..3: needed at the start of round-0 chunk ch.
        for ch, gs in ((1, (12, 13)), (2, (20, 21)), (3, (36, 37))):
            for m, g in zip((0, 1), gs):
                sched[g].append(proj_piece(qT, wq_sb, bq_sb, m, ch, "q"))
        # kT23 (full): needed by round-1 chunk 0 (g=64).
        for (m, ch), g in zip(
            [(m, ch) for ch in range(4) for m in (2, 3)],
            (24, 25, 28, 29, 40, 41, 44, 45),
        ):
            sched[g].append(proj_piece(kT, wk_sb, bk_sb, m, ch, "k"))
        # qT23: ch c needed at the start of round-1 chunk c.
        for ch, gs in ((0, (52, 53)), (1, (60, 61)), (2, (82, 83)), (3, (98, 99))):
            for m, g in zip((2, 3), gs):
                sched[g].append(proj_piece(qT, wq_sb, bq_sb, m, ch, "q"))
        # out-projection: rows lt ready after round-1 chunk lt//4's tail.
        for lt0, g0 in ((0, 84), (4, 100), (8, 116)):
            gg = g0
            for lt in range(lt0, lt0 + 4):
                for cc in (0, 1):
                    sched[gg].append(outproj_piece(lt, cc))
                    gg += 1
        drain = [outproj_piece(lt, cc, act_evac=True) for lt in range(12, 16) for cc in (0, 1)]

        # ---------------- main pipeline ----------------
        for g in range(128):
            c, lk = divmod(g, 16)
            qk_exp(c, lk, g)
            if g >= LAG:
                pv_den(g - LAG)
                if (g - LAG) % 16 == 15:
                    tail((g - LAG) // 16)
            for thunk in sched.pop(g, ()):
                thunk()
        for g2 in range(128 - LAG, 128):
            pv_den(g2)
        tail(7, last=True)
        for thunk in drain:
            thunk()
        assert not sched and not ats_hist

        est.close()

    nc.compile()
    return nc


def _prep_inputs(query, qkv_proj, qkv_bias, out_proj):
    """Per-core input shards (host-side)."""
    query = np.asarray(query, dtype=np.float32)
    qkv_proj = np.asarray(qkv_proj, dtype=np.float32)
    qkv_bias = np.asarray(qkv_bias, dtype=np.float32)
    W3 = qkv_proj.reshape(E, 3, E)  # [i, c, e], row f = 3*i + c
    b3 = qkv_bias.reshape(E, 3)
    bf = ml_dtypes.bfloat16
    maps = []
    for c in range(8):
        n, half = c // 2, c % 2
        isl = slice(IL * half, IL * half + IL)
        maps.append(
            {
                "qt": np.ascontiguousarray(query[:, n, :].T).astype(bf),
                "wq": np.ascontiguousarray(W3[isl, 0, :].T * SCALE).astype(bf),
                "wk": np.ascontiguousarray(W3[isl, 1, :].T).astype(bf),
                "wv": np.ascontiguousarray(W3[isl, 2, :].T).astype(bf),
                "bq": np.ascontiguousarray((b3[isl, 0] * SCALE).reshape(4, P)),
                "bk": np.ascontiguousarray(b3[isl, 1].reshape(4, P)),
                "bvb": np.ascontiguousarray(np.broadcast_to(b3[isl, 2], (P, IL))),
                "opt": np.ascontiguousarray(out_proj[:, isl].T).astype(bf),
            }
        )
    return maps


def kernel(query, qkv_proj, qkv_bias, out_proj, out_bias, **run_kwargs):
    global _built
    out_proj = np.asarray(out_proj, dtype=np.float32)
    out_bias = np.asarray(out_bias, dtype=np.float32)
    if _built is None:
        _built = build()
    in_maps = _prep_inputs(query, qkv_proj, qkv_bias, out_proj)
    res = run_bass_kernel_spmd(_built, in_maps, core_ids=list(range(8)), **run_kwargs)
    parts = [r["out"].astype(np.float32) for r in res.results]
    out = np.empty((L, N, E), dtype=np.float32)
    for n in range(N):
        out[:, n, :] = parts[2 * n] + parts[2 * n + 1] + out_bias
    kernel.last_result = res
    return out


# revision 19
# speedup vs baseline: 1.3764x; 1.1472x over previous
"""Multi-head self-attention (L=2048, N=4, E=1024, h=16) on 8 NeuronCores.

Sharding: core c handles batch n = c//2 and heads [8*(c%2), 8*(c%2)+8).
Each core computes q/k/v projections for its (n, head-block), attention,
and a partial out-projection (columns of out_proj for its heads).
Host sums the two bf16 partials per batch n and adds out_bias.

v2 schedule: one uniform pipeline over 128 global key-steps (8 chunks x
16 lk).  Per step: QK pair + exp per lane; pv/den matmuls lag 3 steps
and carry across chunk boundaries (no boundary stall); each chunk's
normalization tail (strided 4-row reciprocal -> stage -> gpsimd
partition_broadcast -> mults) runs decoupled on DVE/GPSIMD.  The qt
DMA streams in [128,512] pieces (ch0 first) so the first exp fires
~15us in; projection/out-proj work is spread as deadline-scheduled
fillers (v-proj split by round, qT23 + out-proj inside round 1).

Known HW pitfalls encoded here: reciprocal_approx_fast (custom-DVE)
computes garbage via this compile path; partition_broadcast requires
partition-0 sources; GPSIMD cannot access PSUM; interleaved PSUM
accumulation groups need skip_group_check (per-element has_written).
"""

from collections import defaultdict
from contextlib import ExitStack

import ml_dtypes
import numpy as np

import concourse.bacc as bacc
import concourse.mybir as mybir
import concourse.tile as tile
from concourse.bass_utils import run_bass_kernel_spmd

L, N, E, H, D = 2048, 4, 1024, 16, 64
SCALE = D**-0.5
IL = 512  # inner dims per core (8 heads * 64)
P = 128
F32 = mybir.dt.float32
BF16 = mybir.dt.bfloat16
EXP = mybir.ActivationFunctionType.Exp

_built = None


def build(dbg=False):
    nc = bacc.Bacc("TRN2", target_bir_lowering=False, debug=False, num_devices=8)

    qt_d = nc.dram_tensor("qt", [E, L], BF16, kind="ExternalInput")
    wq_d = nc.dram_tensor("wq", [E, IL], BF16, kind="ExternalInput")
    wk_d = nc.dram_tensor("wk", [E, IL], BF16, kind="ExternalInput")
    wv_d = nc.dram_tensor("wv", [E, IL], BF16, kind="ExternalInput")
    bq_d = nc.dram_tensor("bq", [4, P], F32, kind="ExternalInput")
    bk_d = nc.dram_tensor("bk", [4, P], F32, kind="ExternalInput")
    bvb_d = nc.dram_tensor("bvb", [P, IL], F32, kind="ExternalInput")
    opt_d = nc.dram_tensor("opt", [IL, E], BF16, kind="ExternalInput")
    out_d = nc.dram_tensor("out", [L, E], BF16, kind="ExternalOutput")

    with tile.TileContext(nc) as tc:
        est = ExitStack()
        persist = est.enter_context(tc.tile_pool(name="persist", bufs=1))

        ones_col = persist.tile([P, 1], BF16, name="ones_col")
        nc.vector.memset(ones_col, 1.0)
        # Preload the exp table set during the DMA window so the first real
        # EXP doesn't pay the ~2.7us ACT_TABLE_LOAD on the critical path.
        warm = persist.tile([P, 1], F32, name="warm")
        nc.scalar.activation(out=warm, in_=ones_col, func=EXP)

        qT = [persist.tile([P, L], BF16, name=f"qT{m}") for m in range(4)]
        kT = [persist.tile([P, L], BF16, name=f"kT{m}") for m in range(4)]
        vv = [persist.tile([P, IL], BF16, name=f"v{t}") for t in range(16)]
        aoT = [persist.tile([P, L], BF16, name=f"aoT{m}") for m in range(4)]
        opt_sb = [persist.tile([P, E], BF16, name=f"opt{k}") for k in range(4)]
        qt_sb = [persist.tile([P, L], BF16, name=f"qtsb{t}") for t in range(8)]
        wq_sb = [persist.tile([P, IL], BF16, name=f"wq{t}") for t in range(8)]
        wk_sb = [persist.tile([P, IL], BF16, name=f"wk{t}") for t in range(8)]
        wv_sb = [persist.tile([P, IL], BF16, name=f"wv{t}") for t in range(8)]
        bq_sb = persist.tile([P, 4], F32, name="bq_sb")
        bk_sb = persist.tile([P, 4], F32, name="bk_sb")
        bvb_sb = persist.tile([P, IL], F32, name="bvb_sb")

        # ---------------- streaming inputs ----------------
        # Issue order = consumption order: wk + qt-ch0 feed the kT01ch0
        # prologue, wq the qT01ch0 prologue; later qt chunks arrive while
        # the pipeline runs; wv/bvb before the chunk-0 v fillers.
        def dma_qt_ch(ch):
            for t in range(8):
                nc.sync.dma_start(
                    out=qt_sb[t][:, ch * 512 : (ch + 1) * 512],
                    in_=qt_d[t * P : (t + 1) * P, ch * 512 : (ch + 1) * 512],
                )

        for t in range(8):
            nc.sync.dma_start(out=wk_sb[t], in_=wk_d[t * P : (t + 1) * P, :])
            nc.sync.dma_start(
                out=qt_sb[t][:, 0:512], in_=qt_d[t * P : (t + 1) * P, 0:512]
            )
            nc.sync.dma_start(out=wq_sb[t], in_=wq_d[t * P : (t + 1) * P, :])
        for m in range(4):
            nc.sync.dma_start(out=bk_sb[:, m : m + 1], in_=bk_d[m, :, None])
            nc.sync.dma_start(out=bq_sb[:, m : m + 1], in_=bq_d[m, :, None])
        dma_qt_ch(1)
        for t in range(8):
            nc.sync.dma_start(out=wv_sb[t], in_=wv_d[t * P : (t + 1) * P, :])
        nc.sync.dma_start(out=bvb_sb, in_=bvb_d[:, :])
        dma_qt_ch(2)
        dma_qt_ch(3)
        for k in range(4):
            nc.sync.dma_start(out=opt_sb[k], in_=opt_d[k * P : (k + 1) * P, :])

        # ---------------- prologue: kT01 ch0 + qT01 ch0 (DMA-paced) -----
        with tc.tile_pool(name="pro_ps", bufs=1, space="PSUM") as pro_ps:
            psk = {
                m: pro_ps.tile([P, 512], F32, tag=f"pk{m}", name=f"psk{m}")
                for m in (0, 1)
            }
            for t in range(8):
                for m in (0, 1):
                    nc.tensor.matmul(
                        psk[m],
                        wk_sb[t][:, m * P : (m + 1) * P],
                        qt_sb[t][:, 0:512],
                        start=(t == 0),
                        stop=(t == 7),
                    )
            for m in (0, 1):
                nc.vector.tensor_scalar_add(
                    out=kT[m][:, 0:512], in0=psk[m], scalar1=bk_sb[:, m : m + 1]
                )
            psq = {
                m: pro_ps.tile([P, 512], F32, tag=f"pq{m}", name=f"psq{m}")
                for m in (0, 1)
            }
            for t in range(8):
                for m in (0, 1):
                    nc.tensor.matmul(
                        psq[m],
                        wq_sb[t][:, m * P : (m + 1) * P],
                        qt_sb[t][:, 0:512],
                        start=(t == 0),
                        stop=(t == 7),
                    )
            for m in (0, 1):
                nc.vector.tensor_scalar_add(
                    out=qT[m][:, 0:512], in0=psq[m], scalar1=bq_sb[:, m : m + 1]
                )

        # ---------------- pipeline pools ----------------
        ph2 = est.enter_context(ExitStack())
        LAG = 4  # pv lag behind QK/exp, in global steps (den adds up to 3)
        at_pools = [
            ph2.enter_context(tc.tile_pool(name=f"at{i}", bufs=LAG + 4))
            for i in (0, 1)
        ]
        small = ph2.enter_context(tc.tile_pool(name="small", bufs=4))
        osb = ph2.enter_context(tc.tile_pool(name="osb", bufs=3))
        pvc = ph2.enter_context(tc.tile_pool(name="pvc", bufs=4))
        st_ps = [
            ph2.enter_context(tc.tile_pool(name=f"st{i}", bufs=1, space="PSUM"))
            for i in (0, 1)
        ]
        pv_ps = [
            ph2.enter_context(tc.tile_pool(name=f"pv{i}", bufs=1, space="PSUM"))
            for i in (0, 1)
        ]
        den_ps = ph2.enter_context(tc.tile_pool(name="den", bufs=1, space="PSUM"))
        fill_ps = ph2.enter_context(tc.tile_pool(name="fill", bufs=1, space="PSUM"))

        # shared den tile: rows 64i+32j.  One memset arms the never-written
        # rows so the [0:97] whole-span reciprocal reads defined data; the
        # next chunk's den matmuls WAR on that reciprocal, so they are
        # DELAYED (den_hold) until it has drained.
        den_t = den_ps.tile([P, 512], F32, name="den_g")
        nc.vector.memset(den_t, 1.0)

        ats_hist = {}  # global step g -> [at tiles per lane]
        pv_cur = {}  # lane p -> psum tile of the in-flight chunk

        def lanes_of(c):
            return (2 * (c // 4), 2 * (c // 4) + 1)

        def qk_exp(c, lk, g):
            rnd, lq = divmod(c, 4)
            lqs = slice(lq * 512, (lq + 1) * 512)
            lks = slice(lk * P, (lk + 1) * P)
            ats = []
            for i, p in enumerate(lanes_of(c)):
                st = st_ps[i].tile([P, 2, 512], F32, tag="st", name=f"st_{p}_{g}")
                for j in (0, 1):
                    nc.tensor.matmul(
                        st[:, j, :],
                        kT[p][64 * j : 64 * j + 64, lks],
                        qT[p][64 * j : 64 * j + 64, lqs],
                        start=True,
                        stop=True,
                    )
                at = at_pools[i].tile(
                    [P, 2, 512], BF16, tag="at", name=f"at_{p}_{g}"
                )
                nc.scalar.activation(out=at, in_=st, func=EXP)
                ats.append(at)
            ats_hist[g] = ats

        den_hold = []  # (lk2, ats) buffered so den matmuls clear the
        # previous chunk's reciprocal read before first touching den_t

        def den_step(c2, lk2, ats):
            lanes = lanes_of(c2)
            for i, p in enumerate(lanes):
                for j in (0, 1):
                    r0 = 64 * i + 32 * j
                    nc.tensor.matmul(
                        den_t[r0 : r0 + 1, :],
                        ones_col,
                        ats[i][:, j, :],
                        start=(lk2 == 0),
                        stop=(lk2 == 15),
                        tile_position=(0, r0),
                        skip_group_check=True,
                    )

        def pv_den(g2):
            c2, lk2 = divmod(g2, 16)
            lanes = lanes_of(c2)
            ats = ats_hist.pop(g2)
            if lk2 == 0:
                for i, p in enumerate(lanes):
                    pv_cur[p] = pv_ps[i].tile(
                        [P, 512], F32, tag="pv", name=f"pv_{p}_{c2}"
                    )
            for i, p in enumerate(lanes):
                for j in (0, 1):
                    nc.tensor.matmul(
                        pv_cur[p][64 * j : 64 * j + 64, :],
                        vv[lk2][:, P * p + 64 * j : P * p + 64 * j + 64],
                        ats[i][:, j, :],
                        start=(lk2 == 0),
                        stop=(lk2 == 15),
                        skip_group_check=True,
                    )
            if lk2 < 3:
                den_hold.append((lk2, ats))
            else:
                while den_hold:
                    hlk, hats = den_hold.pop(0)
                    den_step(c2, hlk, hats)
                den_step(c2, lk2, ats)

        def tail(c2, last=False):
            rnd, lq = divmod(c2, 4)
            lanes = lanes_of(c2)
            lqs = slice(lq * 512, (lq + 1) * 512)
            # DVE order matters: the pv copies and den-row copies free the
            # PSUM banks the next chunk's pv/den matmuls WAR on, so they
            # must precede the ~3.5us reciprocal in the DVE FIFO.
            if not last:
                pvs = {}
                for i, p in enumerate(lanes):
                    pvs[p] = pvc.tile([P, 512], F32, tag="pvc", name=f"pvc_{p}_{c2}")
                    nc.vector.tensor_copy(out=pvs[p], in_=pv_cur[p])
            else:
                pvs = {p: pv_cur[p] for p in lanes}
            rcp = pvc.tile([P, 512], F32, tag="rcp", name=f"rcp_{c2}", bufs=2)
            nc.vector.reciprocal(out=rcp[0:97, :], in_=den_t[0:97, :])
            for i, p in enumerate(lanes):
                bcs = small.tile(
                    [P, 2, 512], F32, tag="bcs", name=f"bcs_{p}_{c2}", bufs=2
                )
                rc = small.tile(
                    [1, 2, 512], F32, tag="rc", name=f"rc_{p}_{c2}", bufs=2
                )
                # partition_broadcast's ucode reads via Q7 core 0 only,
                # so the source must sit on partition 0 — stage the two
                # reciprocal rows there first.
                for j in (0, 1):
                    r0 = 64 * i + 32 * j
                    nc.vector.tensor_copy(out=rc[:, j, :], in_=rcp[r0 : r0 + 1, :])
                nc.gpsimd.partition_broadcast(bcs, rc)
                for j in (0, 1):
                    nc.vector.tensor_mul(
                        out=aoT[p][64 * j : 64 * j + 64, lqs],
                        in0=pvs[p][64 * j : 64 * j + 64, :],
                        in1=bcs[64 * j : 64 * j + 64, j, :],
                    )

        # ---------------- deadline-scheduled fillers ----------------
        sched = defaultdict(list)

        def v_piece(lk):
            def thunk():
                ps = fill_ps.tile([P, 512], F32, tag="fill", name=f"fv{lk}")
                for t in range(8):
                    nc.tensor.matmul(
                        ps,
                        qt_sb[t][:, lk * P : (lk + 1) * P],
                        wv_sb[t],
                        start=(t == 0),
                        stop=(t == 7),
                    )
                nc.vector.tensor_add(out=vv[lk], in0=ps, in1=bvb_sb)
            return thunk

        def proj_piece(dest, w_sb, bias_sb, m, ch, nm):
            def thunk():
                ps = fill_ps.tile([P, 512], F32, tag="fill", name=f"f{nm}{m}{ch}")
                for t in range(8):
                    nc.tensor.matmul(
                        ps,
                        w_sb[t][:, m * P : (m + 1) * P],
                        qt_sb[t][:, ch * 512 : (ch + 1) * 512],
                        start=(t == 0),
                        stop=(t == 7),
                    )
                nc.vector.tensor_scalar_add(
                    out=dest[m][:, ch * 512 : (ch + 1) * 512],
                    in0=ps,
                    scalar1=bias_sb[:, m : m + 1],
                )
            return thunk

        def outproj_piece(lt, cc, act_evac=False, ps_fn=None):
            def thunk():
                if ps_fn is not None:
                    ps = ps_fn()
                else:
                    ps = fill_ps.tile([P, 512], F32, tag="fill", name=f"fo{lt}{cc}")
                for k in range(4):
                    nc.tensor.matmul(
                        ps,
                        aoT[k][:, lt * P : (lt + 1) * P],
                        opt_sb[k][:, cc * 512 : (cc + 1) * 512],
                        start=(k == 0),
                        stop=(k == 3),
                    )
                ob = osb.tile([P, 512], BF16, tag="ob", name=f"ob{lt}{cc}")
                if act_evac:
                    nc.scalar.copy(out=ob, in_=ps)
                else:
                    nc.vector.tensor_copy(out=ob, in_=ps)
                nc.sync.dma_start(
                    out=out_d[lt * P : (lt + 1) * P, cc * 512 : (cc + 1) * 512],
                    in_=ob,
                )
            return thunk

        # Filler placement rules: meet each consumer's deadline, and keep
        # clear of the tail-issue windows [16c+19, 16c+21] so filler evac
        # adds never queue behind a tail's DVE chain (the next filler's
        # matmuls WAR on the shared fill bank).
        # v projection: piece lk needed by chunk-0 pv_den at g=lk+LAG.
        for lk in range(16):
            sched[lk].append(v_piece(lk))
        # kT01 remaining chunks: ch needed by chunk-0 QK at lk = 4*ch.
        for ch, gs in ((1, (0, 1)), (2, (4, 5)), (3, (8, 9))):
            for m, g in zip((0, 1), gs):
                sched[g].append(proj_piece(kT, wk_sb, bk_sb, m, ch, "k"))
        # qT01 ch1..3: needed at the start of round-0 chunk ch.
        for ch, gs in ((1, (12, 13)), (2, (24, 25)), (3, (40, 41))):
            for m, g in zip((0, 1), gs):
                sched[g].append(proj_piece(qT, wq_sb, bq_sb, m, ch, "q"))
        # kT23 (full): needed by round-1 chunk 0 (g=64).
        for (m, ch), g in zip(
            [(m, ch) for ch in range(4) for m in (2, 3)],
            (26, 27, 30, 31, 44, 45, 56, 57),
        ):
            sched[g].append(proj_piece(kT, wk_sb, bk_sb, m, ch, "k"))
        # qT23: ch c needed at the start of round-1 chunk c.
        for ch, gs in ((0, (58, 59)), (1, (60, 61)), (2, (76, 77)), (3, (92, 93))):
            for m, g in zip((2, 3), gs):
                sched[g].append(proj_piece(qT, wq_sb, bq_sb, m, ch, "q"))
        # out-projection: rows lt ready after round-1 chunk lt//4's tail.
        # act_evac: ScalarE evacuation keeps these pieces' PSUM recycling
        # off the DVE queue, where the chunk tails would cascade-stall them.
        for lt0, g0 in ((0, 86), (4, 102), (8, 118)):
            gg = g0
            for lt in range(lt0, lt0 + 4):
                for cc in (0, 1):
                    sched[gg].append(outproj_piece(lt, cc, act_evac=True))
                    gg += 1
        # drain pieces rotate their PSUM target over the fill bank and the
        # by-then-free den/pv banks, so each piece's matmuls overlap the
        # previous piece's evacuation instead of WAR-serializing on one bank.
        _rot = [None, lambda: den_t, lambda: pv_cur[2], lambda: pv_cur[3]]
        drain = [
            outproj_piece(lt, cc, act_evac=True, ps_fn=_rot[(2 * (lt - 12) + cc) % 4])
            for lt in range(12, 16)
            for cc in (0, 1)
        ]

        # ---------------- main pipeline ----------------
        for g in range(128):
            c, lk = divmod(g, 16)
            qk_exp(c, lk, g)
            if g >= LAG:
                pv_den(g - LAG)
                if (g - LAG) % 16 == 15:
                    tail((g - LAG) // 16)
            for thunk in sched.pop(g, ()):
                thunk()
        for g2 in range(128 - LAG, 128):
            pv_den(g2)
        tail(7, last=True)
        for thunk in drain:
            thunk()
        assert not sched and not ats_hist

        est.close()

    nc.compile()
    return nc


def _prep_inputs(query, qkv_proj, qkv_bias, out_proj):
    """Per-core input shards (host-side)."""
    query = np.asarray(query, dtype=np.float32)
    qkv_proj = np.asarray(qkv_proj, dtype=np.float32)
    qkv_bias = np.asarray(qkv_bias, dtype=np.float32)
    W3 = qkv_proj.reshape(E, 3, E)  # [i, c, e], row f = 3*i + c
    b3 = qkv_bias.reshape(E, 3)
    bf = ml_dtypes.bfloat16
    maps = []
    for c in range(8):
        n, half = c // 2, c % 2
        isl = slice(IL * half, IL * half + IL)
        maps.append(
            {
                "qt": np.ascontiguousarray(query[:, n, :].T).astype(bf),
                "wq": np.ascontiguousarray(W3[isl, 0, :].T * SCALE).astype(bf),
                "wk": np.ascontiguousarray(W3[isl, 1, :].T).astype(bf),
                "wv": np.ascontiguousarray(W3[isl, 2, :].T).astype(bf),
                "bq": np.ascontiguousarray((b3[isl, 0] * SCALE).reshape(4, P)),
                "bk": np.ascontiguousarray(b3[isl, 1].reshape(4, P)),
                "bvb": np.ascontiguousarray(np.broadcast_to(b3[isl, 2], (P, IL))),
                "opt": np.ascontiguousarray(out_proj[:, isl].T).astype(bf),
            }
        )
    return maps


def kernel(query, qkv_proj, qkv_bias, out_proj, out_bias, **run_kwargs):
    global _built
    out_proj = np.asarray(out_proj, dtype=np.float32)
    out_bias = np.asarray(out_bias, dtype=np.float32)
    if _built is None:
        _built = build()
    in_maps = _prep_inputs(query, qkv_proj, qkv_bias, out_proj)
    res = run_bass_kernel_spmd(_built, in_maps, core_ids=list(range(8)), **run_kwargs)
    parts = [r["out"].astype(np.float32) for r in res.results]
    out = np.empty((L, N, E), dtype=np.float32)
    for n in range(N):
        out[:, n, :] = parts[2 * n] + parts[2 * n + 1] + out_bias
    kernel.last_result = res
    return out
